# revision 20
# baseline (speedup 1.0000x reference)
"""Trainium2 Bass kernel for nn_Memory_24438363915056 (scatter_memory).

Reference computation (D = S = 8192, fp32):
    sim     = (k @ MK) / (||k|| * ||MK_col||)        # cosine addressing, (1, S)
    w_r     = softmax(sim)
    min_idx = argmin(w_u)                            # least-used slot
    mk      = MK with column min_idx zeroed
    mu      = MU with column min_idx zeroed
    w_u_new = 0.95 * w_u + w_r + w_w
    beta    = sigmoid(beta_param)
    w_w_new = beta * w_r + (1 - beta) * w_lu
    w_lu_new= (w_u_new < min(w_u_new))  -> all zeros (strict less-than of min)
    mu_new  = mu + w_w_new * u                       # row-broadcast add (D==S)
    u_final = w_r @ mk.T
    mk_new  = mk + w_u_new * k                       # row-broadcast add

Strategy: shard MK/MU column-wise (memory_slots dim) across 8 NeuronCores,
with shards TRANSPOSED on host so slots j sit on SBUF partitions and the
i-axis is the free dim. Two device passes:
  A: t[j] = sum_i k[i]*MK[i,j] and n2[j] = sum_i MK[i,j]^2 as free-axis
     fused multiply-reduce on VectorE (k pre-broadcast across partitions).
  B: mk_newT = MKT + rowk[j] / mu_newT = MUT + rowu[j] as per-partition
     tensor_scalar adds (DVE 2x mode); u_final partials via PE matmuls
     accumulating wr'.T @ mk_newT over slot chunks (host subtracts the
     exact sum_j wr'[j]*rowk[j] correction).
Host does the tiny O(S) glue (softmax normalization across shards, argmin,
usage updates), the single-column min_idx fixup, and layout transposes.

The walrus build here allows at most 2 semaphore waits per instruction (1 for
matmul/LDWEIGHTS); every tile below has a single compute-engine reader or
producer per DMA so Tile's emitted waits stay within that.
"""

import os
import sys
from contextlib import ExitStack

import numpy as np

try:
    import concourse.bass as bass
except ImportError:  # fresh grading dir: fall back to the repo checkout
    sys.path.insert(0, "/opt/trn_rl_repo")
    import concourse.bass as bass

import concourse.tile as tile
from concourse import mybir
from concourse.bass_utils import run_bass_kernel_spmd

D = 8192            # knowledge_dim == understanding_dim == memory_slots
NCORES = 8
SHARD = D // NCORES  # 1024 slots per core
P = 128              # SBUF partitions
GAMMA = 0.95
FP32 = mybir.dt.float32

# filled by kernel() when KERNEL_PROFILE=1: {"a_ns": int, "b_ns": int}
LAST_PROFILE = {}

# The walrus build in this image encodes a limited number of semaphore waits
# per ISA instruction (observed: 1 for Matmult/TensorScalarPtr, 2 for
# DMACopy/Activation/...), while Tile freely emits 3+ (and ~10 on the final
# Drain).  Legalize at BIR-JSON level: hoist excess waits onto injected
# same-engine NoOps immediately before the offending instruction (engines
# execute their stream in order, so this preserves semantics exactly).
_WAIT_LIMITS = {}
_DEFAULT_WAIT_LIMIT = 1
_NOP_WAIT_LIMIT = 1


def _legalize_sync_waits(bir_json: bytes) -> bytes:
    import json

    m = json.loads(bir_json)
    n_nops = [0]
    for fn in m.get("functions", []):
        for blk in fn.get("blocks", []):
            insts = blk.get("instructions")
            if not insts:
                continue
            out = []
            for inst in insts:
                si = inst.get("sync_info")
                waits = (si or {}).get("on_wait") or []
                lim = _WAIT_LIMITS.get(inst.get("opcode"), _DEFAULT_WAIT_LIMIT)
                if len(waits) > lim:
                    excess, keep = waits[: len(waits) - lim], waits[len(waits) - lim :]
                    for i in range(0, len(excess), _NOP_WAIT_LIMIT):
                        out.append(
                            {
                                "debug": inst.get("debug", 0),
                                "engine": inst["engine"],
                                "ins": [],
                                "outs": [],
                                "name": f"I-lgl-{n_nops[0]}",
                                "opcode": "NoOp",
                                "sync_info": {
                                    "on_update": [],
                                    "on_wait": excess[i : i + _NOP_WAIT_LIMIT],
                                },
                            }
                        )
                        n_nops[0] += 1
                    si["on_wait"] = keep
                out.append(inst)
            blk["instructions"] = out
    return json.dumps(m).encode()


def _install_legalizer():
    if getattr(bass.Bass, "_sync_legalizer_installed", False):
        return
    orig = bass.Bass.to_json_bytes
    bass.Bass.to_json_bytes = lambda self: _legalize_sync_waits(orig(self))
    bass.Bass._sync_legalizer_installed = True


_install_legalizer()


def _install_profile_hook():
    """Make run_bass_kernel_spmd(trace=True) work in this container.

    The axon NTFF profile hook normally lives in antenv.axon_hooks, which this
    image lacks; inject a sys.modules shim wired to libaxon_pjrt.so, and stub
    the artifact upload (no bucket access here).
    """
    import types

    import concourse.bass_utils as bu

    if "antenv.axon_hooks" not in sys.modules:
        mod = types.ModuleType("antenv.axon_hooks")
        mod._hook = None
        mod.set_axon_ntff_profile_hook = lambda h: setattr(mod, "_hook", h)
        mod.get_axon_ntff_profile_hook = lambda: mod._hook
        sys.modules["antenv.axon_hooks"] = mod
        sys.path.insert(0, "/root/.axon_site")
        from trn_agent_boot.trn_boot import _ntff_profile_via_ctypes

        mod._hook = _ntff_profile_via_ctypes("/opt/axon/libaxon_pjrt.so")
    bu.upload_artifacts = lambda tmpdir: f"local:{tmpdir}"


def _build_kernel_a(d=D, shard=SHARD):
    """Pass A on the natural shard: t[j] = sum_i k[i]*MK[i,j], n2[j] = ||col j||^2.

    mk_shard: (d, shard).  PE contracts the partition axis: t via k-column
    lhsT, n2 via ones-column lhsT over ACT-squared tiles.  DVE makes private
    copies so each DMA'd tile has exactly one reader engine (walrus allows
    at most 2 sync waits per instruction here, 1 on LDWEIGHTS).
    k_aux: [128, nchunk+1]: col r = k[128r:128r+128], last col = ones.
    """
    nchunk = d // P
    nc = bass.Bass()
    mk = nc.dram_tensor("mk_shard", [d, shard], FP32, kind="ExternalInput")
    k_aux = nc.dram_tensor("k_aux", [P, nchunk + 1], FP32, kind="ExternalInput")
    out_tn = nc.dram_tensor("out_tn", [2, shard], FP32, kind="ExternalOutput")

    nblock = min(512, shard)
    with tile.TileContext(nc) as tc, ExitStack() as ctx:
        rawp = ctx.enter_context(tc.tile_pool(name="rawp", bufs=8))
        sqp = ctx.enter_context(tc.tile_pool(name="sqp", bufs=4))
        smallp = ctx.enter_context(tc.tile_pool(name="smallp", bufs=1))
        psp = ctx.enter_context(
            tc.tile_pool(name="psp", bufs=1, space=bass.MemorySpace.PSUM)
        )

        kc = smallp.tile([P, nchunk + 1], FP32)
        nc.sync.dma_start(kc[:], k_aux[:])
        ones = kc[:, nchunk : nchunk + 1]

        pt = psp.tile([1, shard], FP32)
        pn = psp.tile([1, shard], FP32)

        for r in range(nchunk):
            raw = rawp.tile([P, shard], FP32)
            nc.sync.dma_start(raw[:], mk[r * P : (r + 1) * P, :])
            sq = sqp.tile([P, shard], FP32)
            nc.scalar.activation(sq[:], raw[:], mybir.ActivationFunctionType.Square)
            for b in range(shard // nblock):
                sl = bass.ts(b, nblock)
                nc.tensor.matmul(
                    pt[:, sl], kc[:, r : r + 1], raw[:, sl],
                    start=(r == 0), stop=(r == nchunk - 1),
                )
                nc.tensor.matmul(
                    pn[:, sl], ones[:], sq[:, sl],
                    start=(r == 0), stop=(r == nchunk - 1),
                )

        res_t = smallp.tile([1, shard], FP32)
        nc.vector.tensor_copy(res_t[:], pt[:])
        res_n = smallp.tile([1, shard], FP32)
        nc.vector.tensor_copy(res_n[:], pn[:])
        nc.sync.dma_start(out_tn[0:1, :], res_t[:])
        nc.sync.dma_start(out_tn[1:2, :], res_n[:])
    return nc


def _build_kernel_b(d=D, shard=SHARD, fblk=1024):
    """Pass B on transposed shards: memory updates + u_final partials.

    aux_col: [128, 3*njc]: cols [0:njc] wr' (masked w_r), [njc:2njc] rowk,
    [2njc:3njc] rowu, laid out so col jc holds slots jc*128..jc*128+127.
    upart[0, i] = sum_{local j} wr'[j] * (mkT[j, i] + rowk[j]); host subtracts
    sum_j wr'[j]*rowk[j].
    """
    njc = shard // P
    nib = d // fblk
    nc = bass.Bass()
    mkT = nc.dram_tensor("mkT_shard", [shard, d], FP32, kind="ExternalInput")
    muT = nc.dram_tensor("muT_shard", [shard, d], FP32, kind="ExternalInput")
    aux = nc.dram_tensor("aux_col", [P, 3 * njc], FP32, kind="ExternalInput")
    mk_new = nc.dram_tensor("mk_newT", [shard, d], FP32, kind="ExternalOutput")
    mu_new = nc.dram_tensor("mu_newT", [shard, d], FP32, kind="ExternalOutput")
    upart = nc.dram_tensor("upart", [1, d], FP32, kind="ExternalOutput")

    with tile.TileContext(nc) as tc, ExitStack() as ctx:
        rawk = ctx.enter_context(tc.tile_pool(name="rawk", bufs=4))
        mkop = ctx.enter_context(tc.tile_pool(name="mkop", bufs=4))
        rawu = ctx.enter_context(tc.tile_pool(name="rawu", bufs=4))
        muop = ctx.enter_context(tc.tile_pool(name="muop", bufs=4))
        smallp = ctx.enter_context(tc.tile_pool(name="smallp", bufs=1))
        psp = ctx.enter_context(
            tc.tile_pool(name="psp", bufs=2, space=bass.MemorySpace.PSUM)
        )

        auxt = smallp.tile([P, 3 * njc], FP32)
        nc.sync.dma_start(auxt[:], aux[:])

        ub = smallp.tile([1, d], FP32)

        for ib in range(nib):
            pu = psp.tile([1, fblk], FP32)
            for jc in range(njc):
                rows = slice(jc * P, (jc + 1) * P)
                cols = slice(ib * fblk, (ib + 1) * fblk)
                t = rawk.tile([P, fblk], FP32)
                nc.sync.dma_start(t[:], mkT[rows, cols])
                mko = mkop.tile([P, fblk], FP32)
                nc.vector.tensor_scalar(
                    out=mko[:], in0=t[:],
                    scalar1=auxt[:, njc + jc : njc + jc + 1], scalar2=None,
                    op0=mybir.AluOpType.add,
                )
                nc.sync.dma_start(mk_new[rows, cols], mko[:])
                # u_part over mko = mkT + rowk; host subtracts sum(wr'*rowk)
                for h in range(fblk // 512):
                    hs = bass.ts(h, 512)
                    nc.tensor.matmul(
                        pu[:, hs], auxt[:, jc : jc + 1], mko[:, hs],
                        start=(jc == 0), stop=(jc == njc - 1),
                    )

                m = rawu.tile([P, fblk], FP32)
                nc.sync.dma_start(m[:], muT[rows, cols])
                muo = muop.tile([P, fblk], FP32)
                nc.vector.tensor_scalar(
                    out=muo[:], in0=m[:],
                    scalar1=auxt[:, 2 * njc + jc : 2 * njc + jc + 1], scalar2=None,
                    op0=mybir.AluOpType.add,
                )
                nc.sync.dma_start(mu_new[rows, cols], muo[:])

            nc.vector.tensor_copy(ub[:, ib * fblk : (ib + 1) * fblk], pu[:])

        nc.sync.dma_start(upart[:, :], ub[:, :])
    return nc


def kernel(k, u, memory_knowledge, memory_understanding, w_w, w_u, w_lu, beta_param):
    profile = bool(int(os.environ.get("KERNEL_PROFILE", "0")))
    if profile:
        _install_profile_hook()
    run_kwargs = dict(trace=True) if profile else {}

    k = np.asarray(k, dtype=np.float32)
    u = np.asarray(u, dtype=np.float32)
    MK = np.asarray(memory_knowledge, dtype=np.float32)
    MU = np.asarray(memory_understanding, dtype=np.float32)
    w_w = np.asarray(w_w, dtype=np.float32)
    w_u = np.asarray(w_u, dtype=np.float32)
    w_lu = np.asarray(w_lu, dtype=np.float32)
    beta_param = np.asarray(beta_param, dtype=np.float32)

    cores = list(range(NCORES))
    njc = SHARD // P
    MKT = np.ascontiguousarray(MK.T)  # (slots, D)
    MUT = np.ascontiguousarray(MU.T)
    mkT_shards = [MKT[c * SHARD : (c + 1) * SHARD] for c in cores]
    muT_shards = [MUT[c * SHARD : (c + 1) * SHARD] for c in cores]
    mk_shards = [
        np.ascontiguousarray(MK[:, c * SHARD : (c + 1) * SHARD]) for c in cores
    ]

    # ---- pass A: per-slot dot with k, and squared column norms ----
    nchunk = D // P
    k_aux = np.ones((P, nchunk + 1), dtype=np.float32)
    k_aux[:, :nchunk] = k.reshape(nchunk, P).T  # col r = k[128r:128r+128]
    nc_a = _build_kernel_a()
    res_a = run_bass_kernel_spmd(
        nc_a,
        [{"mk_shard": mk_shards[c], "k_aux": k_aux} for c in cores],
        cores,
        **run_kwargs,
    )
    if profile:
        LAST_PROFILE["a_ns"] = res_a.exec_time_ns

    t = np.concatenate([r["out_tn"][0] for r in res_a.results])  # (D,)
    n2 = np.concatenate([r["out_tn"][1] for r in res_a.results])  # (D,)

    # ---- host glue: O(S) vector math ----
    sim = t / (np.linalg.norm(k) * np.sqrt(n2))
    e = np.exp(sim - sim.max())
    w_r = (e / e.sum()).astype(np.float32)  # (D,)
    min_idx = int(np.argmin(w_u))

    w_u_new = (GAMMA * w_u + w_r + w_w).astype(np.float32)
    beta = 1.0 / (1.0 + np.exp(-float(beta_param)))
    w_w_new = (beta * w_r + (1.0 - beta) * w_lu).astype(np.float32)
    w_lu_new = np.where(w_u_new < w_u_new.min(), 1.0, 0.0).astype(np.float32)

    rowk = (w_u_new * k[0]).astype(np.float32)  # (D,)
    rowu = (w_w_new * u[0]).astype(np.float32)  # (D,)
    wrm = w_r.copy()
    wrm[min_idx] = 0.0  # zeroed column drops out of u_final

    # ---- pass B: memory updates + u_final partials ----
    nc_b = _build_kernel_b()
    in_b = []
    for c in cores:
        aux = np.empty((P, 3 * njc), dtype=np.float32)
        sl = slice(c * SHARD, (c + 1) * SHARD)
        aux[:, 0:njc] = wrm[sl].reshape(njc, P).T
        aux[:, njc : 2 * njc] = rowk[sl].reshape(njc, P).T
        aux[:, 2 * njc : 3 * njc] = rowu[sl].reshape(njc, P).T
        in_b.append(
            {"mkT_shard": mkT_shards[c], "muT_shard": muT_shards[c], "aux_col": aux}
        )
    res_b = run_bass_kernel_spmd(nc_b, in_b, cores, **run_kwargs)
    if profile:
        LAST_PROFILE["b_ns"] = res_b.exec_time_ns

    mk_new = np.vstack([r["mk_newT"] for r in res_b.results]).T.copy()
    mu_new = np.vstack([r["mu_newT"] for r in res_b.results]).T.copy()
    u_final = np.zeros(D, dtype=np.float32)
    for c in cores:
        u_final += res_b.results[c]["upart"][0]
    # device reduced over mko = mkT + rowk; remove the folded-in rowk term
    u_final -= np.float32(np.dot(wrm.astype(np.float64), rowk.astype(np.float64)))

    # min_idx column was zeroed before the broadcast add
    mk_new[:, min_idx] = rowk[min_idx]
    mu_new[:, min_idx] = rowu[min_idx]

    return (
        u_final.reshape(1, D),
        mk_new,
        mu_new,
        w_w_new.reshape(1, D),
        w_u_new.reshape(1, D),
        w_lu_new.reshape(1, D),
    )
